# revision 30
# baseline (speedup 1.0000x reference)
"""BitConv2dInfer on 8 Trainium2 NeuronCores — fp8 DoubleRow edition.

Reference computation (per full input):
    x = clip(x, -1, 1)                       # x [32, 256, 56, 56] f32
    y = conv2d(x, w_q, pad=1)                # w_q [256, 256, 3, 3] ternary
    y = y * s + bias                         # per-out-channel affine
Sharding: data-parallel over batch — each of the 8 cores gets 4 images and
the full (tiny) weights; outputs concatenate over batch with no comms.

Numerics: the ternary weights are exact in fp8-e4m3, and the clamped
activations quantize to e4m3 with ~1.3% RMS noise; end-to-end max relative
error is 1.58e-2 (bit-identical to a CPU simulation of e4m3-quantized
activations), inside the 2e-2 gate. This unlocks the PE's DoubleRow mode:
2 fp8 weights per cell virtualize the array to a 256-deep contraction, so
the full CIN=256 reduces in ONE pass and the 3x3 conv needs 9 accumulated
matmuls per output chunk (vs 18 for bf16). Measured DoubleRow issue rate
is ~464 cycles per 464-column matmul — the fp8 roofline. (An
element-interleaved pair layout measured 20% slower per matmul; the
plane-separated layout with pair stride = plane stride is the fast one.)

Device kernel (per core, per image):
  - x[n] DMAs in as [128, 2(ci), 56*56] f32, split across the gpsimd /
    scalar / sync rings; image n+1's loads are issued BEFORE image n's
    matmul work so output traffic never queues ahead of them
  - DVE clamps to [-1,1] + casts to fp8 into a zero-bordered flat pad
    tile [128, 2, 3376] (58x58 rows, plane stride 16-aligned), one clamp
    per arriving piece so conversion pipelines with the DMA
  - conv = 9 DoubleRow matmuls per 8-row chunk over a FLAT window:
      rhs = xpad[:, :, (g0+kh)*58+kw : +464]  (crosses row boundaries;
      the 2 wrap columns per row land in PSUM columns never evacuated)
    taps outermost over 4/3-chunk halves so one 256-column LDWEIGHTS
    serves 3-4 matmuls
  - the DoubleRow rhs spans both ci planes, so the dependency tracker
    gates those matmuls on the ENTIRE pad tile. For image 0 (loaded
    row-chunked) the first two chunks instead run as 2x9 single-plane
    non-DoubleRow matmuls — per-plane windows carry precise row deps, so
    the PE starts at ~11.5us while the image is still streaming in, and
    stays busy (HAM never re-throttles) until the full-image gate clears
  - scalar ACT evacuates PSUM (cols 0..55 of each row) with per-channel
    scale+bias; the final tile's evacuation splits across ACT + DVE and
    closing DMAs are small so the tail drains fast
  - outputs rotate over sync/scalar/gpsimd rings, balanced to ~9MB/ring

The PE clock gate (HAM) starts at 1.2 GHz and reaches 2.4 GHz only after
~3.4us of sustained activity, so dummy matmuls front-run while the first
input chunks are in flight.

Weights are host-side transposed to lhsT layout [128 cin, co, tap, ci,
cout] and cast to fp8e4 (exact for ternary values).
"""

import sys

sys.path.insert(0, "/opt/trn_rl_repo")

import ml_dtypes
import numpy as np

import concourse.bass as bass  # noqa: F401  (registers engines)
import concourse.mybir as mybir
import concourse.tile as tile
from concourse import bacc
from concourse.bass_utils import run_bass_kernel_spmd

N, CIN, COUT, H, W = 32, 256, 256, 56, 56
NCORES = 8
NB = N // NCORES          # images per core
WP = W + 2                # padded row length
PLANE = 3376              # 58*58=3364 rounded up to a 16B multiple
RG = 8                    # output rows per PSUM chunk (8*58=464 <= 512 f32/bank)
L = RG * WP               # flat matmul window length
NCH = H // RG             # chunks per image
NCI = CIN // 128          # cin tiles (paired via DoubleRow)
NCO = COUT // 128         # cout tiles
NTAP = 9
N_NDR = 1                 # leading image-0 chunks via non-DoubleRow matmuls
# First-image input chunk schedule, (engine, ci, row0, nrows) in issue
# order: three rings (scalar/sync HWDGE first byte ~8.6us, gpsimd SWDGE
# ~10.5us) deliver rows ahead of consumption.
N0_CHUNKS = [
    ("s", 0, 0, 9), ("y", 1, 0, 9),
    ("s", 0, 9, 8), ("y", 1, 9, 8),
    ("s", 1, 17, 8), ("y", 0, 17, 8),
    ("g", 0, 25, 8), ("g", 1, 25, 8),
    ("s", 0, 33, 8), ("y", 1, 33, 8),
    ("g", 0, 41, 8), ("g", 1, 41, 8),
    ("s", 0, 49, 7), ("y", 1, 49, 7),
]
N_WARM_MM = 22            # dummy matmuls to lift the HAM clock gate

_compiled = {}


def _build():
    nc = bacc.Bacc("TRN2", target_bir_lowering=False, debug=False)
    f32, bf16, f8 = mybir.dt.float32, mybir.dt.bfloat16, mybir.dt.float8e4
    DR = mybir.MatmulPerfMode.DoubleRow
    x_d = nc.dram_tensor("x", [NB, CIN, H, W], f32, kind="ExternalInput").ap()
    w_d = nc.dram_tensor(
        "w", [128, NCO, NTAP, NCI, 128], f8, kind="ExternalInput"
    ).ap()
    sb_d = nc.dram_tensor("sb", [128, 2 * NCO], f32, kind="ExternalInput").ap()
    o_d = nc.dram_tensor("out", [NB, COUT, H, W], f32, kind="ExternalOutput").ap()

    clamp = dict(op0=mybir.AluOpType.max, op1=mybir.AluOpType.min)
    affine = dict(op0=mybir.AluOpType.mult, op1=mybir.AluOpType.add)

    with tile.TileContext(nc) as tc:
        with (
            tc.tile_pool(name="const", bufs=1) as cpool,
            tc.tile_pool(name="xs", bufs=2) as xspool,
            tc.tile_pool(name="xsc", bufs=7) as xscpool,
            tc.tile_pool(name="xpad", bufs=2) as xppool,
            tc.tile_pool(name="osb", bufs=3) as opool,
            tc.tile_pool(name="ps", bufs=7, space="PSUM") as pspool,
            tc.tile_pool(name="warmps", bufs=1, space="PSUM") as wpspool,
        ):
            w_sb = cpool.tile([128, NCO, NTAP, NCI, 128], f8, tag="w")
            sb_sb = cpool.tile([128, 2 * NCO], f32, tag="sb")

            # HAM pre-warm. Memset on gpsimd so the vector queue stays
            # clear for the border memsets + clamps gating the first MMs.
            warm = cpool.tile([128, RG * W], bf16, tag="warm")
            nc.gpsimd.memset(warm[:], 0.0)
            warm_ps = wpspool.tile([128, RG * W], f32, tag="warmps")
            for _ in range(N_WARM_MM):
                nc.tensor.matmul(
                    out=warm_ps[:], lhsT=warm[:, 0:128], rhs=warm[:],
                    start=True, stop=True,
                )

            def pad_tile(eng):
                """Pad tile + border memsets; win[p,ci,r,c] = padded (r+1,c+1)."""
                xp = xppool.tile([128, NCI, PLANE], f8, tag="xpad", name="xp")
                grid = xp[:, :, 0 : 58 * WP].rearrange(
                    "p t (h w) -> p t h w", w=WP
                )
                win = grid[:, :, 1:57, 1:57]
                eng.memset(xp[:, :, 0:WP], 0.0)
                eng.memset(xp[:, :, 57 * WP : PLANE], 0.0)
                # col 57 of row r and col 0 of row r+1 are adjacent
                strip = xp[:, :, WP - 1 : WP - 1 + 57 * WP].rearrange(
                    "p t (h w) -> p t h w", w=WP
                )
                eng.memset(strip[:, :, :, 0:2], 0.0)
                return xp, win

            # First image, row-chunked. Only the tiny taps-0/1 weight piece
            # rides ahead of the first x pieces on scalar; the rest of co=0's
            # weights go via gpsimd, whose image-0 rows are late-deadline —
            # anything bulky at the head of the scalar/sync rings delays the
            # chunk-0/1 clamps that gate the first real matmuls.
            nc.scalar.dma_start(out=w_sb[:, 0, 0:2], in_=w_d[:, 0, 0:2])
            nc.gpsimd.dma_start(out=w_sb[:, 0, 2:9], in_=w_d[:, 0, 2:9])
            nc.gpsimd.dma_start(out=sb_sb[:], in_=sb_d)
            n0_xp, n0_win = pad_tile(nc.vector)
            engs = {"g": nc.gpsimd, "s": nc.scalar, "y": nc.sync}
            n0_stage = []
            for eng_key, ci, r0, nr in N0_CHUNKS:
                xsc = xscpool.tile([128, 9, W], f32, tag=f"xsc{ci}", name="xsc")
                engs[eng_key].dma_start(
                    out=xsc[:, 0:nr],
                    in_=x_d[0, ci * 128 : (ci + 1) * 128, r0 : r0 + nr],
                )
                n0_stage.append((r0, nr, ci, xsc))
            nc.sync.dma_start(out=w_sb[:, 1], in_=w_d[:, 1])
            for r0, nr, ci, xsc in n0_stage:
                nc.vector.tensor_scalar(
                    n0_win[:, ci : ci + 1, r0 : r0 + nr],
                    xsc[:, 0:nr], -1.0, 1.0, **clamp,
                )

            # Prefetch rides only the gpsimd + sync rings: their engine
            # queues hold nothing deadline-critical, so a token-gated DMA
            # issue stalling there is harmless. (Scheduled onto scalar it
            # lands between ACT evacuations — the scheduler's cost model
            # thinks the token clears early — and a stalled scalar queue
            # blocks PSUM recycling: measured 7.3us PE gap.)
            PIECES = ((0, 0, H, "g"), (1, 0, H, "y"))

            def prefetch_dma(n, gate):
                """Issue image n's loads (3 rings) into a staging tile.

                The clamps happen later, in convert() at the top of image
                n's own block: the vector queue is in-order, and a clamp
                stalling on a still-streaming DMA there would block every
                later clamp behind it (measured as a 19.5us PE gap when
                prefetch clamps interleaved with image 0's chunk clamps).

                `gate` is an element the previous image's LAST clamp wrote.
                The 1-element token copies read it and write each piece's
                corner, so every load DMA takes a WAW dep on a token that
                cannot run (true data dep — the scheduler can't hoist it)
                before the previous image is fully staged. Without this
                the big prefetch packets win the SDMA engines'
                packet-granularity round-robin and starve the small
                deadline pieces of the image being computed (measured
                ~55-70 GB/s on those rings vs 186 on the prefetch ring).
                """
                xs = xspool.tile([128, NCI, H * W], f32, tag="xs")
                xs4 = xs[:].rearrange("p t (h w) -> p t h w", w=W)
                for ci, r0, r1, _ in PIECES:
                    nc.vector.tensor_copy(
                        out=xs4[:, ci, r0 : r0 + 1, 0:1], in_=gate
                    )
                for ci, r0, r1, ek in PIECES:
                    engs[ek].dma_start(
                        out=xs4[:, ci, r0:r1],
                        in_=x_d[n, ci * 128 : (ci + 1) * 128, r0:r1],
                    )
                return xs4

            def convert(xs4):
                """Pad tile + piecewise clamp of a staged (landed) image."""
                xp, win = pad_tile(nc.vector)
                for ci, r0, r1, _ in PIECES:
                    nc.vector.tensor_scalar(
                        win[:, ci : ci + 1, r0:r1],
                        xs4[:, ci, r0:r1], -1.0, 1.0, **clamp,
                    )
                return xp, win

            def conv_groups(xp, co, chunk_sets, osb4, taper_last):
                """Tap-outermost DoubleRow matmuls + ACT evacuation."""
                for chunks in chunk_sets:
                    ndr = chunks[0] < N_NDR and co == 0 and xp is n0_xp
                    pss = {
                        c: pspool.tile([128, L], f32, tag="ps", name=f"ps{c}")
                        for c in chunks
                    }
                    for t in range(NTAP):
                        kh, kw = divmod(t, 3)
                        for c in chunks:
                            off = (c * RG + kh) * WP + kw
                            if ndr:
                                # Per-plane windows: precise row-range deps
                                # let these start mid-load of image 0.
                                for ci in range(NCI):
                                    nc.tensor.matmul(
                                        out=pss[c][:],
                                        lhsT=w_sb[:, co, t, ci],
                                        rhs=xp[:, ci, off : off + L],
                                        start=(t == 0 and ci == 0),
                                        stop=(t == NTAP - 1 and ci == NCI - 1),
                                    )
                            else:
                                nc.tensor.matmul(
                                    out=pss[c][:],
                                    lhsT=w_sb[:, co, t],
                                    rhs=xp[:, :, off : off + L],
                                    start=(t == 0),
                                    stop=(t == NTAP - 1),
                                    perf_mode=DR,
                                )
                    for c in chunks:
                        ps4 = pss[c][:].rearrange("p (h w) -> p h w", w=WP)
                        scale = sb_sb[:, co : co + 1]
                        bias = sb_sb[:, NCO + co : NCO + co + 1]
                        if taper_last and c == NCH - 1:
                            # Final evacuation split across ACT + DVE so the
                            # two halves drain in parallel.
                            nc.scalar.activation(
                                out=osb4[:, c * RG : c * RG + 4, :],
                                in_=ps4[:, 0:4, 0:W],
                                func=mybir.ActivationFunctionType.Identity,
                                bias=bias, scale=scale,
                            )
                            nc.vector.tensor_scalar(
                                osb4[:, c * RG + 4 : c * RG + 8, :],
                                ps4[:, 4:8, 0:W], scale, bias, **affine,
                            )
                        else:
                            nc.scalar.activation(
                                out=osb4[:, c * RG : c * RG + RG, :],
                                in_=ps4[:, 0:RG, 0:W],
                                func=mybir.ActivationFunctionType.Identity,
                                bias=bias, scale=scale,
                            )

            xp_cur = n0_xp
            xs_next = prefetch_dma(1, n0_win[:, 1:2, 55:56, 0:1])
            for n in range(NB):
                if n > 0:
                    xp_cur, win_cur = convert(xs_next)
                    xs_next = (
                        prefetch_dma(n + 1, win_cur[:, 1:2, 55:56, 0:1])
                        if n + 1 < NB
                        else None
                    )
                for co in range(NCO):
                    last_tile = n == NB - 1 and co == NCO - 1
                    if co == 0:
                        out_eng = nc.sync if n < NB - 1 else nc.scalar
                    else:
                        out_eng = nc.scalar
                    osb = opool.tile([128, H * W], f32, tag="osb")
                    osb4 = osb[:].rearrange("p (h w) -> p h w", w=W)
                    if n == 0 and co == 0:
                        # Chunk-sequential, paced with the row-chunk DMAs.
                        chunk_sets = [(c,) for c in range(NCH)]
                    elif last_tile:
                        # Half then singles so the tail evacuations overlap
                        # the remaining chunks' matmuls.
                        chunk_sets = [(0, 1, 2, 3), (4,), (5,), (6,)]
                    else:
                        chunk_sets = [(0, 1, 2, 3), (4, 5, 6)]
                    conv_groups(xp_cur, co, chunk_sets, osb4, last_tile)
                    dst = o_d[n, co * 128 : (co + 1) * 128]
                    if last_tile:
                        out_eng.dma_start(out=dst[:, 0:32], in_=osb4[:, 0:32])
                        out_eng.dma_start(out=dst[:, 32:48], in_=osb4[:, 32:48])
                        nc.sync.dma_start(out=dst[:, 48:52], in_=osb4[:, 48:52])
                        nc.scalar.dma_start(out=dst[:, 52:56], in_=osb4[:, 52:56])
                    else:
                        out_eng.dma_start(out=dst[:, 0:32], in_=osb4[:, 0:32])
                        out_eng.dma_start(out=dst[:, 32:H], in_=osb4[:, 32:H])

    nc.compile()
    return nc


def _prep_weights(w_q, s, bias):
    # lhsT layout: [cin_k (128 partitions), co, tap, ci, cout_j] so that
    # w_t[k, co, t, ci, j] = w_q[co*128 + j, ci*128 + k, kh, kw]
    w_t = (
        w_q.astype(np.float32)
        .transpose(2, 3, 1, 0)                 # [kh, kw, CIN, COUT]
        .reshape(NTAP, NCI, 128, NCO, 128)     # [tap, ci, k, co, j]
        .transpose(2, 3, 0, 1, 4)              # [k, co, tap, ci, j]
        .astype(ml_dtypes.float8_e4m3)
    )
    sb_t = np.concatenate(
        [
            np.ascontiguousarray(s.reshape(NCO, 128).T.astype(np.float32)),
            np.ascontiguousarray(bias.reshape(NCO, 128).T.astype(np.float32)),
        ],
        axis=1,
    )
    return np.ascontiguousarray(w_t), np.ascontiguousarray(sb_t)


def kernel(x, w_q, s, bias):
    if "nc" not in _compiled:
        _compiled["nc"] = _build()
    nc = _compiled["nc"]

    w_t, sb_t = _prep_weights(w_q, s, bias)
    x = np.ascontiguousarray(x, dtype=np.float32)
    core_ids = list(range(NCORES))
    in_maps = [
        {"x": x[i * NB : (i + 1) * NB], "w": w_t, "sb": sb_t}
        for i in core_ids
    ]
    res = run_bass_kernel_spmd(nc, in_maps, core_ids)
    return np.concatenate([res.results[i]["out"] for i in core_ids], axis=0)
